# revision 5
# baseline (speedup 1.0000x reference)
"""Bass/Trainium2 kernel for nn_EpisodeMultiheadAttentionBlock.

Reference computation (B=4, L=4096, E=256, Q=2048):
    x  = key[:, -Q:]
    q  = rope(x @ Wq.T + bq, idx[:, -Q:]);  k = rope(key @ Wk.T + bk, idx)
    v  = key @ Wv.T + bv
    s  = (q / sqrt(E)) @ k.T  with causal mask (bottom Q rows of triu)
    w  = softmax(s)                            -> output 2
    y  = (w @ v) @ Wo.T + bo
    r  = sigmoid(x@Wxr.T + y@Wyr.T); z = sigmoid(x@Wxz.T + y@Wyz.T)
    hh = tanh((r*x)@Wxg.T + y@Wyg.T)
    out = (1-z)*x + z*hh                       -> output 1

Sharding: 8 cores = 4 batches x 2 query-interleavings. Core (b, h) owns
query 128-row blocks g = 2m+h (m=0..7) of batch b. The interleaving makes
the causal key extent per local block m uniform across cores: E(m) = 18+2m
key blocks (even-parity cores carry one fully-masked pad block), so one
SPMD program serves all 8 cores; the parity-dependent boundary mask is a
tiny per-core input added into the score PSUM via an identity matmul.

Layouts: projections/gating run in "T layout" (feature dim on partitions)
so every matmul's operands arrive in the orientation the next matmul
consumes. RoPE's pair rotation becomes a between-subtile elementwise op by
permuting the feature dim of Wq/Wk to [evens | odds] host-side (the
permutation cancels inside q.k). Scores are computed naturally
([query-part, key-free]) so the softmax row-sum falls out of the scalar
engine's exp accum_out and w DMAs straight out; the attention matmul's
j-on-partitions operand is built with PE transposes of the exp'd tiles.
"""

import sys

sys.path.insert(0, "/opt/trn_rl_repo")

import numpy as np
import ml_dtypes

import concourse.bass as bass
import concourse.tile as tile
from concourse import bacc, mybir
from concourse.bass_utils import run_bass_kernel_spmd
from concourse.masks import make_identity

F32 = mybir.dt.float32
F32R = mybir.dt.float32r
BF16 = mybir.dt.bfloat16
BF16_NP = np.dtype(ml_dtypes.bfloat16)

B, L, E, Q = 4, 4096, 256, 2048
P = 128
NM = 8              # query 128-blocks per core
NEG = -30000.0      # additive mask; exp(NEG/16) underflows to exactly 0.0
SCALE = 1.0 / 16.0  # 1/sqrt(E)

IdF = mybir.ActivationFunctionType.Identity
ExpF = mybir.ActivationFunctionType.Exp
SigF = mybir.ActivationFunctionType.Sigmoid
TanhF = mybir.ActivationFunctionType.Tanh

_CACHE = {}


def _ext_blocks(m):
    """Causal key extent (in 128-col blocks) for local query block m."""
    return 18 + 2 * m


def build_nc():
    nc = bacc.Bacc("TRN2", target_bir_lowering=False, debug=False,
                   enable_asserts=False, num_devices=8)

    io = {}
    io["keyT"] = nc.dram_tensor("keyT", [2, P, L], F32,
                                kind="ExternalInput").ap()
    io["xT_in"] = nc.dram_tensor("xT_in", [2, P, Q // 2], F32,
                                 kind="ExternalInput").ap()
    io["cosT"] = nc.dram_tensor("cosT", [P, L], BF16,
                                kind="ExternalInput").ap()
    io["sinT"] = nc.dram_tensor("sinT", [P, L], BF16,
                                kind="ExternalInput").ap()
    io["qcs_in"] = nc.dram_tensor("qcs_in", [2, P, Q // 2], BF16,
                                  kind="ExternalInput").ap()
    for n in ["wq", "wk", "wv", "wo", "wxr", "wyr", "wxz", "wyz", "wxg",
              "wyg"]:
        io[n] = nc.dram_tensor(n, [2, P, E], F32, kind="ExternalInput").ap()
    io["mtail"] = nc.dram_tensor("mtail", [P, 2 * P], BF16,
                                 kind="ExternalInput").ap()
    io["biasv"] = nc.dram_tensor("biasv", [2, P, 6], F32,
                                 kind="ExternalInput").ap()
    io["bvbc"] = nc.dram_tensor("bvbc", [P, E], F32,
                                kind="ExternalInput").ap()
    io["w_out"] = nc.dram_tensor("w_out", [NM, P, L], F32,
                                 kind="ExternalOutput").ap()
    io["o_out"] = nc.dram_tensor("o_out", [NM, P, E], F32,
                                 kind="ExternalOutput").ap()

    with tile.TileContext(nc) as tc:
        _emit(nc, tc, io)
    nc.compile()
    return nc


def _emit(nc, tc, io):
    from contextlib import ExitStack

    with ExitStack() as ctx:
        const = ctx.enter_context(tc.tile_pool(name="const", bufs=1))
        big = ctx.enter_context(tc.tile_pool(name="big", bufs=1))
        # single PSUM pool layout (8 banks total):
        #   mmps [P,1024] x2bufs = 4 banks, tpps [P,512] x2 = 2,
        #   atps [P,256] x2 = 2
        mmps = ctx.enter_context(tc.tile_pool(name="mmps", bufs=2,
                                              space="PSUM"))
        tpps = ctx.enter_context(tc.tile_pool(name="tpps", bufs=2,
                                              space="PSUM"))
        atps = ctx.enter_context(tc.tile_pool(name="atps", bufs=2,
                                              space="PSUM"))

        def mm_tile(cols=1024):
            return mmps.tile([P, 1024], F32, name="mmt", tag="mm")[:, :cols]

        # ---- constants -----------------------------------------------
        ident = const.tile([P, P], F32)
        make_identity(nc, ident)
        identb = const.tile([P, P], BF16)
        make_identity(nc, identb)
        mtail = const.tile([P, 2 * P], BF16)
        nc.sync.dma_start(mtail[:], io["mtail"])
        biasv = const.tile([P, 2, 6], F32)
        for s in range(2):
            nc.sync.dma_start(biasv[:, s, :], io["biasv"][s])
        bvbc = const.tile([P, E], F32)
        nc.sync.dma_start(bvbc[:], io["bvbc"])
        zeros = const.tile([P, 1792], F32)
        nc.vector.memset(zeros[:], 0.0)
        wt = {}
        with tc.tile_pool(name="stgp", bufs=2) as stgp:
            for n in ["wq", "wk", "wv", "wo", "wxr", "wyr", "wxz", "wyz",
                      "wxg", "wyg"]:
                stg = stgp.tile([P, 2, E], F32, name="stg", tag="stg")
                for s in range(2):
                    nc.sync.dma_start(stg[:, s, :], io[n][s])
                wt[n] = const.tile([P, 2, E], F32R, name=f"wt_{n}")
                nc.vector.tensor_copy(wt[n][:], stg[:])

        # ---- long-lived activations ----------------------------------
        v = big.tile([P, L // P, E], F32R)         # 32KB/part
        xT = big.tile([P, 2, Q // 2], F32)         # 8KB
        xTr = big.tile([P, 2, Q // 2], F32R)       # 8KB (rounded twin)
        attn = big.tile([P, NM, E], F32)           # 8KB
        recips = big.tile([P, NM], F32)
        for s in range(2):
            nc.sync.dma_start(xT[:, s, :], io["xT_in"][s])
        nc.vector.tensor_copy(xTr[:], xT[:])

        with tc.tile_pool(name="rotp", bufs=1) as rotp:
            kTrot = rotp.tile([P, 2, L], BF16)     # 16KB
            qTrot = rotp.tile([P, 2, Q // 2], BF16)

            # ---- projections + rope (scoped pools) -------------------
            with tc.tile_pool(name="keyp", bufs=1) as keyp, \
                 tc.tile_pool(name="csp", bufs=1) as csp, \
                 tc.tile_pool(name="ropet", bufs=3) as ropet:
                keyT = keyp.tile([P, 2, L], F32R)  # 32KB
                for s in range(2):
                    for c0 in range(0, L, 1024):
                        ks = keyp.tile([P, 1024], F32, name="ks", tag="ks",
                                       bufs=3)
                        nc.sync.dma_start(ks[:],
                                          io["keyT"][s, :, c0:c0 + 1024])
                        nc.vector.tensor_copy(keyT[:, s, c0:c0 + 1024],
                                              ks[:])
                cosT = csp.tile([P, L], BF16)
                sinT = csp.tile([P, L], BF16)
                nc.sync.dma_start(cosT[:], io["cosT"])
                nc.sync.dma_start(sinT[:], io["sinT"])
                qcs = csp.tile([P, 2, Q // 2], BF16)
                for s in range(2):
                    nc.sync.dma_start(qcs[:, s, :], io["qcs_in"][s])

                # v[j,d] = sum_e keyT[e,j] WvT[e,d]
                for j4 in range(L // P // 4):
                    vps = mm_tile().rearrange("p (j d) -> p j d", d=E)
                    for jj in range(4):
                        J = j4 * 4 + jj
                        for s in range(2):
                            nc.tensor.matmul(
                                vps[:, jj, :],
                                keyT[:, s, J * P:(J + 1) * P],
                                wt["wv"][:, s, :],
                                start=(s == 0), stop=(s == 1))
                    nc.vector.tensor_copy(v[:, j4 * 4:(j4 + 1) * 4, :],
                                          vps[:])

                def project_rot(dst, src_rhs, wn_, cos_ap, sin_ap, bcol, n):
                    """dst[:,0,:] = a*c - b*s ; dst[:,1,:] = a*s + b*c
                    with (a,b) = halves of (W @ src + bias)."""
                    for c0 in range(0, n, 1024):
                        cw = min(1024, n - c0)
                        pa = mm_tile(cw)
                        pb = mm_tile(cw)
                        for dh, ps in ((0, pa), (1, pb)):
                            for s in range(2):
                                for q0 in range(0, cw, 512):
                                    qw = min(512, cw - q0)
                                    nc.tensor.matmul(
                                        ps[:, q0:q0 + qw],
                                        wt[wn_][:, s, dh * P:(dh + 1) * P],
                                        src_rhs[:, s, c0 + q0:c0 + q0 + qw],
                                        start=(s == 0), stop=(s == 1))
                        ab = ropet.tile([P, 2, 1024], BF16,
                                        name="ab", tag="ab")[:, :, :cw]
                        nc.scalar.activation(ab[:, 0, :], pa[:], IdF,
                                             bias=biasv[:, 0, bcol:bcol + 1])
                        nc.scalar.activation(ab[:, 1, :], pb[:], IdF,
                                             bias=biasv[:, 1, bcol:bcol + 1])
                        c_ap = cos_ap[:, c0:c0 + cw]
                        s_ap = sin_ap[:, c0:c0 + cw]
                        t1 = ropet.tile([P, 1024], BF16, name="t1",
                                        tag="t1")[:, :cw]
                        t2 = ropet.tile([P, 1024], BF16, name="t2",
                                        tag="t2")[:, :cw]
                        nc.vector.tensor_mul(t1[:], ab[:, 0, :], c_ap)
                        nc.gpsimd.tensor_mul(t2[:], ab[:, 1, :], s_ap)
                        nc.vector.tensor_tensor(dst[:, 0, c0:c0 + cw],
                                                t1[:], t2[:],
                                                mybir.AluOpType.subtract)
                        t3 = ropet.tile([P, 1024], BF16, name="t3",
                                        tag="t3")[:, :cw]
                        t4 = ropet.tile([P, 1024], BF16, name="t4",
                                        tag="t4")[:, :cw]
                        nc.gpsimd.tensor_mul(t3[:], ab[:, 0, :], s_ap)
                        nc.vector.tensor_mul(t4[:], ab[:, 1, :], c_ap)
                        nc.vector.tensor_tensor(dst[:, 1, c0:c0 + cw],
                                                t3[:], t4[:],
                                                mybir.AluOpType.add)

                project_rot(kTrot, keyT, "wk", cosT, sinT, 1, L)
                project_rot(qTrot, xTr, "wq", qcs[:, 0, :], qcs[:, 1, :],
                            0, Q // 2)

            # ---- main attention loop ---------------------------------
            with tc.tile_pool(name="expp", bufs=6) as expp, \
                 tc.tile_pool(name="wnp", bufs=3) as wnp, \
                 tc.tile_pool(name="wtp", bufs=4) as wtp, \
                 tc.tile_pool(name="smallp", bufs=2) as smallp:
                for m in range(NM):
                    EB = _ext_blocks(m)
                    ext = EB * P
                    nch = (ext + 1023) // 1024
                    parts = smallp.tile([P, 4], F32, name="parts",
                                        tag="parts")
                    aps = atps.tile([P, E], F32, name="aps", tag="aps")
                    exp_tiles = []
                    jdone = 0
                    for c in range(nch):
                        c0 = c * 1024
                        cw = min(1024, ext - c0)
                        sps = mm_tile(cw)
                        for q0 in range(0, cw, 512):
                            qw = min(512, cw - q0)
                            last = (c == nch - 1) and (q0 + qw == cw)
                            for s in range(2):
                                nc.tensor.matmul(
                                    sps[:, q0:q0 + qw],
                                    qTrot[:, s, m * P:(m + 1) * P],
                                    kTrot[:, s, c0 + q0:c0 + q0 + qw],
                                    start=(s == 0),
                                    stop=(s == 1) and not last)
                            if last:
                                nc.tensor.matmul(
                                    sps[:, cw - 2 * P:cw], identb[:],
                                    mtail[:], start=False, stop=True,
                                    skip_group_check=True)
                        ex = expp.tile([P, 1024], F32, name="ex",
                                       tag="ex")[:, :cw]
                        nc.scalar.activation(ex[:], sps[:], ExpF, bias=0.0,
                                             scale=SCALE,
                                             accum_out=parts[:, c:c + 1])
                        exp_tiles.append((ex, c0, cw))
                        for g0 in range(0, cw, 512):
                            gw = min(512, cw - g0)
                            nblk = gw // P
                            tp = tpps.tile([P, 512], F32, name="tp",
                                           tag="tp")[:, :gw]
                            for jj in range(nblk):
                                nc.tensor.transpose(
                                    tp[:, jj * P:(jj + 1) * P],
                                    ex[:, g0 + jj * P:g0 + (jj + 1) * P],
                                    ident[:])
                            wTs = wtp.tile([P, 512], F32R, name="wTs",
                                           tag="wTs")[:, :gw]
                            nc.vector.tensor_copy(wTs[:], tp[:])
                            for jj in range(nblk):
                                J = jdone + jj
                                nc.tensor.matmul(
                                    aps[:],
                                    wTs[:, jj * P:(jj + 1) * P],
                                    v[:, J, :],
                                    start=(J == 0), stop=(J == EB - 1))
                            jdone += nblk
                    sig = smallp.tile([P, 1], F32, name="sig", tag="sig")
                    nc.vector.tensor_reduce(sig[:], parts[:, :nch],
                                            mybir.AxisListType.X,
                                            mybir.AluOpType.add)
                    nc.vector.reciprocal(recips[:, m:m + 1], sig[:])
                    for (ex, c0, cw) in exp_tiles:
                        wn = wnp.tile([P, 1024], F32, name="wn",
                                      tag="wn")[:, :cw]
                        nc.vector.tensor_scalar_mul(wn[:], ex[:],
                                                    recips[:, m:m + 1])
                        nc.sync.dma_start(io["w_out"][m, :, c0:c0 + cw],
                                          wn[:])
                    if ext < L:
                        nc.sync.dma_start(io["w_out"][m, :, ext:L],
                                          zeros[:, :L - ext])
                    nc.vector.tensor_scalar_mul(attn[:, m, :], aps[:],
                                                recips[:, m:m + 1])
                    nc.vector.tensor_add(attn[:, m, :], attn[:, m, :],
                                         bvbc[:])

        # ---- gating (T layout) ---------------------------------------
        with tc.tile_pool(name="gatep", bufs=1) as gatep:
            attnT = gatep.tile([P, 2, Q // 2], F32R)
            for dh in range(2):
                for m4 in range(2):
                    tp = tpps.tile([P, 512], F32, name="tpg", tag="tp")
                    for mm in range(4):
                        m = m4 * 4 + mm
                        nc.tensor.transpose(
                            tp[:, mm * P:(mm + 1) * P],
                            attn[:, m, dh * P:(dh + 1) * P],
                            ident[:])
                    nc.vector.tensor_copy(
                        attnT[:, dh, m4 * 512:(m4 + 1) * 512], tp[:])

            def mm_T(dst_sb, terms, bcol, act):
                """dst[:,dh,:] = act(sum_t W_t.T-lhsT @ rhs_t + bias)."""
                for dh in range(2):
                    pr = mm_tile()
                    ng = len(terms)
                    for gi, (wn_, rhs) in enumerate(terms):
                        for s in range(2):
                            for q0 in range(0, Q // 2, 512):
                                nc.tensor.matmul(
                                    pr[:, q0:q0 + 512],
                                    wt[wn_][:, s, dh * P:(dh + 1) * P],
                                    rhs[:, s, q0:q0 + 512],
                                    start=(gi == 0 and s == 0),
                                    stop=(gi == ng - 1 and s == 1))
                    nc.scalar.activation(dst_sb[:, dh, :], pr[:], act,
                                         bias=biasv[:, dh, bcol:bcol + 1])

            yT = gatep.tile([P, 2, Q // 2], F32)
            mm_T(yT, [("wo", attnT)], 5, IdF)
            yTr = gatep.tile([P, 2, Q // 2], F32R)
            nc.vector.tensor_copy(yTr[:], yT[:])
            rT = gatep.tile([P, 2, Q // 2], F32)
            mm_T(rT, [("wxr", xTr), ("wyr", yTr)], 2, SigF)
            zT = gatep.tile([P, 2, Q // 2], F32)
            mm_T(zT, [("wxz", xTr), ("wyz", yTr)], 3, SigF)

            gx = gatep.tile([P, 2, Q // 2], F32R)
            for dh in range(2):
                nc.vector.tensor_mul(gx[:, dh, :], rT[:, dh, :],
                                     xT[:, dh, :])
            hT = gatep.tile([P, 2, Q // 2], F32)
            mm_T(hT, [("wyg", yTr), ("wxg", gx)], 4, TanhF)

            outT = gatep.tile([P, 2, Q // 2], F32)
            d1p = gatep.tile([P, 2, Q // 2], F32)
            for dh in range(2):
                d1 = d1p[:, dh, :]
                nc.gpsimd.tensor_tensor(d1, hT[:, dh, :], xT[:, dh, :],
                                        mybir.AluOpType.subtract)
                nc.gpsimd.tensor_mul(d1, zT[:, dh, :], d1)
                nc.vector.tensor_add(outT[:, dh, :], xT[:, dh, :], d1)

            onat = gatep.tile([P, NM, E], F32)
            for m4 in range(2):
                for dh in range(2):
                    tp = tpps.tile([P, 512], F32, name="tpo", tag="tp")
                    for mm in range(4):
                        m = m4 * 4 + mm
                        nc.tensor.transpose(
                            tp[:, mm * P:(mm + 1) * P],
                            outT[:, dh, m * P:(m + 1) * P],
                            ident[:])
                    for mm in range(4):
                        m = m4 * 4 + mm
                        dst = onat[:, m, dh * P:(dh + 1) * P]
                        src = tp[:, mm * P:(mm + 1) * P]
                        if (m4 + dh) % 2 == 0:
                            nc.vector.tensor_copy(dst, src)
                        else:
                            nc.scalar.copy(dst, src)
            for m in range(NM):
                nc.sync.dma_start(io["o_out"][m], onat[:, m, :])


# ======================================================================
# host side
# ======================================================================

def _prep_inputs(key, key_index, weights, biases):
    """Build the 8 per-core input dicts."""
    perm = np.concatenate([np.arange(0, E, 2), np.arange(1, E, 2)])
    inv = (1.0 / (np.float32(10000.0) **
                  (np.arange(0, E, 2).astype(np.float32) / np.float32(E))))
    inv = inv.astype(np.float32)

    stair = np.where(np.arange(P)[None, :] <= np.arange(P)[:, None],
                     np.float32(0.0), np.float32(NEG))
    mt = {
        0: np.concatenate([stair, np.full((P, P), NEG, np.float32)],
                          axis=1).astype(BF16_NP),
        1: np.concatenate([np.zeros((P, P), np.float32), stair],
                          axis=1).astype(BF16_NP),
    }

    wq, wk, wv, wo, wxr, wyr, wxz, wyz, wxg, wyg = weights
    bq, bk, bv, bo, bxr, byr, bxz, byz, bxg, byg = biases

    def lhsT(w):
        return np.ascontiguousarray(w.T).reshape(2, P, E)

    w_in = {
        "wq": lhsT(wq[perm]), "wk": lhsT(wk[perm]), "wv": lhsT(wv),
        "wo": lhsT(wo), "wxr": lhsT(wxr), "wyr": lhsT(wyr),
        "wxz": lhsT(wxz), "wyz": lhsT(wyz), "wxg": lhsT(wxg),
        "wyg": lhsT(wyg),
    }
    biasv = np.ascontiguousarray(
        np.stack([bq[perm], bk[perm], bxr + byr, bxz + byz, bxg + byg, bo],
                 axis=1).reshape(2, P, 6)).astype(np.float32)
    bvbc = np.broadcast_to(bv, (P, E)).astype(np.float32).copy()

    in_maps = []
    for b in range(B):
        keyT = np.ascontiguousarray(key[b].T).reshape(2, P, L)
        ang = key_index[b].astype(np.float32)[None, :] * inv[:, None]
        cosT = np.cos(ang).astype(BF16_NP)
        sinT = np.sin(ang).astype(BF16_NP)
        for h in range(2):
            qcols = (L - Q) + np.arange(Q).reshape(16, P)[h::2].reshape(-1)
            xT = np.ascontiguousarray(key[b][qcols].T).reshape(2, P, Q // 2)
            qcs = np.ascontiguousarray(
                np.stack([cosT[:, qcols], sinT[:, qcols]]))
            in_maps.append(dict(
                keyT=keyT, cosT=cosT, sinT=sinT, mtail=np.asarray(mt[h]),
                biasv=biasv, bvbc=bvbc, xT_in=xT, qcs_in=qcs, **w_in))
    return in_maps


def kernel(key, key_index, Wq, bq, Wk, bk, Wv, bv, Wo, bo,
           Wxr, bxr, Wyr, byr, Wxz, bxz, Wyz, byz, Wxg, bxg, Wyg, byg,
           query_length, **extra):
    key = np.asarray(key, np.float32)
    key_index = np.asarray(key_index)
    assert int(query_length) == Q and key.shape == (B, L, E)

    if "nc" not in _CACHE:
        _CACHE["nc"] = build_nc()
    nc = _CACHE["nc"]

    weights = [np.asarray(w, np.float32) for w in
               (Wq, Wk, Wv, Wo, Wxr, Wyr, Wxz, Wyz, Wxg, Wyg)]
    biases = [np.asarray(x, np.float32) for x in
              (bq, bk, bv, bo, bxr, byr, bxz, byz, bxg, byg)]
    in_maps = _prep_inputs(key, key_index, weights, biases)

    res = run_bass_kernel_spmd(nc, in_maps, core_ids=list(range(8)),
                               **_CACHE.get("run_kwargs", {}))
    out = np.empty((B, Q, E), np.float32)
    w = np.empty((B, Q, L), np.float32)
    for core in range(8):
        b, h = divmod(core, 2)
        r = res.results[core]
        out.reshape(B, 16, P, E)[b, h::2] = r["o_out"]
        w.reshape(B, 16, P, L)[b, h::2] = r["w_out"]
    _CACHE["last_result"] = res
    return out, w
